# revision 7
# baseline (speedup 1.0000x reference)
"""Trainium2 Bass kernel for nn_ByteBitwiseFFN.

Reference semantics (per token, D=128 features):
  a = argmax(x[4:20]) + 16*argmax(x[20:36])
  b = argmax(x[36:52]) + 16*argmax(x[52:68])
  res = AND/OR/XOR LUT[a,b] picked by flags x[1]>0.5 / x[2]>0.5 / x[3]>0.5
        (priority AND, OR, XOR; XOR value also used when no flag set)
  active = (x[0]>=0.5) & any-flag; w = active ? 2 : 0
  out = x; out[68 + (res&15)] += w; out[84 + (res>>4)] += w

Key identities:
* Bitwise ops factor over nibbles, so the 256x256 LUTs are never needed:
  res&15 = op(a_lo, b_lo), res>>4 = op(a_hi, b_hi), and for 4-bit operands
  op(u, v) = alpha*(u+v) + beta*(u AND v) with (alpha, beta) =
  (0,1) AND / (1,-1) OR / (1,-2) XOR.  The AND is one int16 bitwise_and.
* Compare-free first-occurrence argmax via the bf16 bit pattern:
  d = max - x >= 0, and for non-negative bf16 the raw bit pattern is
  order-preserving with bits(0) == 0 and bits(d>0) >= 128 (values below
  1e-38 cannot occur: data gaps are > 1e-6).  So
  min over the field of (bitcast_i16(d) + n) == the argmax position n,
  computed entirely in int16.
* One-hot scatter: idx = 16*t + res for active tokens, negative when
  inactive; gpsimd.local_scatter ignores negative indices and zeroes the
  rest, building the +2.0 one-hot plane that is then added into x.

Sharding: pure data parallel over tokens; each of the 8 cores gets
131072/8 = 16384 tokens as its own ExternalInput.

Engine split (from per-op HW microbenchmarks): DVE does the two
reductions, the cand add, flag compares, and the int16 token algebra
(int16 dodges DVE's strided-bf16 slow path); GpSimd does the big
(max - x) f32 subtract, the one-hot local_scatters, and the f32
accumulates; DMA moves contiguous 2MB chunks.
"""

import sys

if "/opt/trn_rl_repo" not in sys.path:
    sys.path.insert(0, "/opt/trn_rl_repo")

import numpy as np

B, S, D = 16, 8192, 128
N_CORES = 8
TOK = B * S                      # 131072 tokens
TOK_PER_CORE = TOK // N_CORES    # 16384
P = 128                          # SBUF partitions

OUT_LO, OUT_HI = 68, 84


def build_program(tok_per_core=TOK_PER_CORE, t_per_chunk=32, group=2):
    """Build + compile the single-core SPMD Bass program.

    The core's [tok_per_core, 128] slab is processed in chunks of 128*T
    tokens (contiguous DRAM block <-> SBUF tile [128, T*128]).  Heavy
    streaming passes run per chunk; small per-token algebra runs once per
    group of `group` chunks.
    """
    import concourse.bass as bass  # noqa: F401
    from concourse import bacc, mybir, tile

    f32 = mybir.dt.float32
    bf16 = mybir.dt.bfloat16
    i16 = mybir.dt.int16
    Op = mybir.AluOpType
    X = mybir.AxisListType.X

    T = t_per_chunk
    chunk_tok = P * T
    assert tok_per_core % (chunk_tok * group) == 0
    n_groups = tok_per_core // (chunk_tok * group)
    GT = group * T                     # tokens-per-partition in one group
    assert GT * 16 * 32 < 2 ** 16      # local_scatter scratch limit

    nc = bacc.Bacc(
        "TRN2",
        target_bir_lowering=False,
        debug=False,
        enable_asserts=True,
        num_devices=N_CORES,
    )
    x_dram = nc.dram_tensor("x", [tok_per_core, D], f32, kind="ExternalInput").ap()
    y_dram = nc.dram_tensor("y", [tok_per_core, D], f32, kind="ExternalOutput").ap()

    with tile.TileContext(nc) as tc:
        with (
            tc.tile_pool(name="consts", bufs=1) as cpool,
            tc.tile_pool(name="xtiles", bufs=4) as xpool,
            tc.tile_pool(name="big", bufs=3) as bp,
            tc.tile_pool(name="small", bufs=2) as sp,
        ):
            v = nc.vector
            g = nc.gpsimd

            # --- constants -------------------------------------------------
            # idxi_full[p, a*16 + n] = n   (for the cand add)
            idxi_full = cpool.tile([P, T * 64], i16)
            nc.gpsimd.iota(idxi_full[:].rearrange("p (a n) -> p a n", n=16),
                           [[0, T * 4], [1, 16]], base=0, channel_multiplier=0)
            # tveci[p, t] = 16*t          (scatter position base)
            tveci = cpool.tile([P, GT], i16)
            nc.gpsimd.iota(tveci[:], [[16, GT]], base=0, channel_multiplier=0)
            # scatter payload: constant 2.0
            data2 = cpool.tile([P, GT], bf16)
            v.memset(data2[:], 2.0)

            for gi in range(n_groups):
                xts = []
                # group result tiles (interleaved [t, 4]), int16
                am_all = sp.tile([P, GT * 4], i16, name="am_all")
                am4 = am_all.rearrange("p (t g) -> p t g", g=4)
                fl_all = sp.tile([P, GT * 4], i16, name="fl_all")
                fl4 = fl_all.rearrange("p (t f) -> p t f", f=4)

                for ci in range(group):
                    i = gi * group + ci
                    xt = xpool.tile([P, T * D], f32, name="xt")
                    xts.append(xt)
                    src = x_dram[i * chunk_tok : (i + 1) * chunk_tok, :].rearrange(
                        "(p t) f -> p (t f)", p=P
                    )
                    nc.sync.dma_start(xt[:], src)

                    x3 = xt.rearrange("p (t f) -> p t f", f=D)
                    nib = x3[:, :, 4:68].rearrange("p t (g n) -> p t g n", n=16)

                    # field max (exact, f32)
                    rmax = bp.tile([P, T * 4], f32, name="rmax")
                    rmax3 = rmax.rearrange("p (t g) -> p t g", g=4)
                    v.tensor_reduce(rmax3, nib, axis=X, op=Op.max)

                    # d = max - x >= 0, as bf16 (GpSimd's fast f32 path)
                    dsub = bp.tile([P, T * 64], bf16, name="dsub")
                    dsub4 = dsub.rearrange("p (t g n) -> p t g n", g=4, n=16)
                    g.tensor_tensor(
                        dsub4,
                        rmax3.unsqueeze(3).broadcast_to([P, T, 4, 16]),
                        nib,
                        Op.subtract,
                    )

                    # cand = bits(d) + n  (int16; 0 exactly at max positions)
                    cand = bp.tile([P, T * 64], i16, name="cand")
                    v.tensor_tensor(
                        cand[:], dsub[:].bitcast(i16), idxi_full[:], Op.add
                    )

                    # per-field argmax position, int16 (first occurrence)
                    v.tensor_reduce(
                        am4[:, ci * T : (ci + 1) * T, :],
                        cand.rearrange("p (t g n) -> p t g n", g=4, n=16),
                        axis=X,
                        op=Op.min,
                    )
                    # flags (>= 0.5) for cols 0..3 as int16 0/1
                    v.tensor_scalar(
                        fl4[:, ci * T : (ci + 1) * T, :],
                        x3[:, :, 0:4],
                        0.5,
                        None,
                        Op.is_ge,
                    )

                # --- per-token algebra for the group, all int16 ------------
                mk = fl4[:, :, 0:1]
                ia = fl4[:, :, 1:2]
                io = fl4[:, :, 2:3]
                ix = fl4[:, :, 3:4]

                def t1(nm):
                    t_ = sp.tile([P, GT], i16, name=nm)
                    return t_.unsqueeze(2)   # [P, GT, 1]

                alpha = t1("alpha")          # 1 - is_and
                v.tensor_scalar(alpha, ia, -1.0, 1.0, Op.mult, Op.add)
                s1 = t1("s1")                # 3 - is_or
                v.tensor_scalar(s1, io, -1.0, 3.0, Op.mult, Op.add)
                s3 = t1("s3")                # is_or - 2
                v.tensor_scalar(s3, io, -2.0, None, Op.add)
                s2 = t1("s2")
                v.tensor_tensor(s2, ia, s1, Op.mult)
                beta = t1("beta")            # 1 / -1 / -2
                v.tensor_tensor(beta, s2, s3, Op.add)
                or1 = t1("or1")
                v.tensor_tensor(or1, ia, io, Op.bitwise_or)
                or2 = t1("or2")
                v.tensor_tensor(or2, or1, ix, Op.bitwise_or)
                acti = t1("acti")            # active = mark & any-flag
                v.tensor_tensor(acti, mk, or2, Op.bitwise_and)
                # pre = 16*t + (active ? 0 : -2048)
                hmk = t1("hmk")
                v.tensor_scalar(hmk, acti, 2048.0, -2048.0, Op.mult, Op.add)
                pre = t1("pre")
                v.tensor_tensor(pre, hmk, tveci.unsqueeze(2), Op.add)

                # per half (lo: fields 0&2, hi: fields 1&3)
                idxs = []
                for h in range(2):
                    qi = t1(f"qi{h}")        # a AND b
                    v.tensor_tensor(
                        qi, am4[:, :, h : h + 1], am4[:, :, h + 2 : h + 3],
                        Op.bitwise_and,
                    )
                    ss = t1(f"ss{h}")        # a + b
                    v.tensor_tensor(
                        ss, am4[:, :, h : h + 1], am4[:, :, h + 2 : h + 3], Op.add
                    )
                    c1 = t1(f"c1{h}")
                    v.tensor_tensor(c1, ss, alpha, Op.mult)
                    c2 = t1(f"c2{h}")
                    v.tensor_tensor(c2, qi, beta, Op.mult)
                    res = t1(f"res{h}")      # op(a, b), 0..15
                    v.tensor_tensor(res, c1, c2, Op.add)
                    idxh = t1(f"idx{h}")     # scatter index (neg = skip)
                    v.tensor_tensor(idxh, res, pre, Op.add)
                    idxs.append(idxh)

                # --- one-hot planes via local_scatter (GpSimd) -------------
                e2s = []
                for h in range(2):
                    e2 = sp.tile([P, GT * 16], bf16, name=f"e2{h}")
                    g.local_scatter(
                        e2[:], data2[:], idxs[h].squeeze(2),
                        channels=P, num_elems=GT * 16, num_idxs=GT,
                    )
                    e2s.append(e2.rearrange("p (t n) -> p t n", n=16))

                # --- accumulate into x and store, per chunk ----------------
                for ci in range(group):
                    i = gi * group + ci
                    x3 = xts[ci].rearrange("p (t f) -> p t f", f=D)
                    for h, off in enumerate((OUT_LO, OUT_HI)):
                        xs = x3[:, :, off : off + 16]
                        g.tensor_tensor(
                            xs, xs, e2s[h][:, ci * T : (ci + 1) * T, :], Op.add
                        )
                    dst = y_dram[i * chunk_tok : (i + 1) * chunk_tok, :].rearrange(
                        "(p t) f -> p (t f)", p=P
                    )
                    nc.sync.dma_start(dst, xts[ci][:])

    nc.compile()
    return nc


_compiled = None


def _get_compiled():
    global _compiled
    if _compiled is None:
        _compiled = build_program()
    return _compiled


def run_on_hw(nc, shards, trace=False, **kw):
    from concourse.bass_utils import run_bass_kernel_spmd

    return run_bass_kernel_spmd(
        nc, [{"x": s} for s in shards], list(range(N_CORES)), trace=trace, **kw
    )


def kernel(x_bd, and_table=None, or_table=None, xor_table=None):
    x = np.ascontiguousarray(np.asarray(x_bd, dtype=np.float32)).reshape(TOK, D)
    shards = [
        np.ascontiguousarray(x[c * TOK_PER_CORE : (c + 1) * TOK_PER_CORE])
        for c in range(N_CORES)
    ]
    nc = _get_compiled()
    res = run_on_hw(nc, shards)
    out = np.concatenate([res.results[c]["y"] for c in range(N_CORES)], axis=0)
    return out.reshape(B, S, D).astype(np.float32)


# revision 9
# speedup vs baseline: 1.1438x; 1.1438x over previous
"""Trainium2 Bass kernel for nn_ByteBitwiseFFN.

Reference semantics (per token, D=128 features):
  a = argmax(x[4:20]) + 16*argmax(x[20:36])
  b = argmax(x[36:52]) + 16*argmax(x[52:68])
  res = AND/OR/XOR LUT[a,b] picked by flags x[1]>0.5 / x[2]>0.5 / x[3]>0.5
        (priority AND, OR, XOR; XOR value also used when no flag set)
  active = (x[0]>=0.5) & any-flag; w = active ? 2 : 0
  out = x; out[68 + (res&15)] += w; out[84 + (res>>4)] += w

Key identities:
* Bitwise ops factor over nibbles, so the 256x256 LUTs are never needed:
  res&15 = op(a_lo, b_lo), res>>4 = op(a_hi, b_hi), and for 4-bit operands
  op(u, v) = alpha*(u+v) + beta*(u AND v) with (alpha, beta) =
  (0,1) AND / (1,-1) OR / (1,-2) XOR.  The AND is one int16 bitwise_and.
* Compare-free first-occurrence argmax via the bf16 bit pattern:
  d = max - x >= 0, and for non-negative bf16 the raw bit pattern is
  order-preserving with bits(0) == 0 and bits(d>0) >= 128 (values below
  1e-38 cannot occur: data gaps are > 1e-6).  So
  min over the field of (bitcast_i16(d) + n) == the argmax position n,
  computed entirely in int16.
* One-hot scatter: idx = 16*t + res for active tokens, negative when
  inactive; gpsimd.local_scatter ignores negative indices and zeroes the
  rest, building the +2.0 one-hot plane that is then added into x.

Sharding: pure data parallel over tokens; each of the 8 cores gets
131072/8 = 16384 tokens as its own ExternalInput.

Engine split (from per-op HW microbenchmarks): DVE does the two
reductions, the cand add, flag compares, and the int16 token algebra
(int16 dodges DVE's strided-bf16 slow path); GpSimd does the big
(max - x) f32 subtract, the one-hot local_scatters, and the f32
accumulates; DMA moves contiguous 2MB chunks.
"""

import sys

if "/opt/trn_rl_repo" not in sys.path:
    sys.path.insert(0, "/opt/trn_rl_repo")

import numpy as np

B, S, D = 16, 8192, 128
N_CORES = 8
TOK = B * S                      # 131072 tokens
TOK_PER_CORE = TOK // N_CORES    # 16384
P = 128                          # SBUF partitions

OUT_LO, OUT_HI = 68, 84


def build_program(tok_per_core=TOK_PER_CORE, t_per_chunk=32, group=2):
    """Build + compile the single-core SPMD Bass program.

    The core's [tok_per_core, 128] slab is processed in chunks of 128*T
    tokens (contiguous DRAM block <-> SBUF tile [128, T*128]).  Heavy
    streaming passes run per chunk; small per-token algebra runs once per
    group of `group` chunks.
    """
    import concourse.bass as bass  # noqa: F401
    from concourse import bacc, mybir, tile

    f32 = mybir.dt.float32
    bf16 = mybir.dt.bfloat16
    i16 = mybir.dt.int16
    Op = mybir.AluOpType
    X = mybir.AxisListType.X

    T = t_per_chunk
    chunk_tok = P * T
    assert tok_per_core % (chunk_tok * group) == 0
    n_groups = tok_per_core // (chunk_tok * group)
    GT = group * T                     # tokens-per-partition in one group
    assert GT * 16 * 32 < 2 ** 16      # local_scatter scratch limit

    nc = bacc.Bacc(
        "TRN2",
        target_bir_lowering=False,
        debug=False,
        enable_asserts=True,
        num_devices=N_CORES,
    )
    x_dram = nc.dram_tensor("x", [tok_per_core, D], f32, kind="ExternalInput").ap()
    y_dram = nc.dram_tensor("y", [tok_per_core, D], f32, kind="ExternalOutput").ap()

    with tile.TileContext(nc) as tc:
        with (
            tc.tile_pool(name="consts", bufs=1) as cpool,
            tc.tile_pool(name="xtiles", bufs=4) as xpool,
            tc.tile_pool(name="big", bufs=3) as bp,
            tc.tile_pool(name="small", bufs=2) as sp,
        ):
            v = nc.vector
            g = nc.gpsimd

            # --- constants -------------------------------------------------
            # idxi_full[p, a*16 + n] = n   (cand add + one-hot compares)
            idxi_full = cpool.tile([P, T * 64], i16)
            nc.gpsimd.iota(idxi_full[:].rearrange("p (a n) -> p a n", n=16),
                           [[0, T * 4], [1, 16]], base=0, channel_multiplier=0)

            for gi in range(n_groups):
                xts = []
                # group result tiles (interleaved [t, 4]), int16
                am_all = sp.tile([P, GT * 4], i16, name="am_all")
                am4 = am_all.rearrange("p (t g) -> p t g", g=4)
                fl_all = sp.tile([P, GT * 4], i16, name="fl_all")
                fl4 = fl_all.rearrange("p (t f) -> p t f", f=4)

                for ci in range(group):
                    i = gi * group + ci
                    xt = xpool.tile([P, T * D], f32, name="xt")
                    xts.append(xt)
                    src = x_dram[i * chunk_tok : (i + 1) * chunk_tok, :].rearrange(
                        "(p t) f -> p (t f)", p=P
                    )
                    nc.sync.dma_start(xt[:], src)

                    x3 = xt.rearrange("p (t f) -> p t f", f=D)
                    nib = x3[:, :, 4:68].rearrange("p t (g n) -> p t g n", n=16)

                    # field max (exact, f32)
                    rmax = bp.tile([P, T * 4], f32, name="rmax")
                    rmax3 = rmax.rearrange("p (t g) -> p t g", g=4)
                    v.tensor_reduce(rmax3, nib, axis=X, op=Op.max)

                    # d = max - x >= 0, as bf16 (GpSimd's fast f32 path)
                    dsub = bp.tile([P, T * 64], bf16, name="dsub")
                    dsub4 = dsub.rearrange("p (t g n) -> p t g n", g=4, n=16)
                    g.tensor_tensor(
                        dsub4,
                        rmax3.unsqueeze(3).broadcast_to([P, T, 4, 16]),
                        nib,
                        Op.subtract,
                    )

                    # cand = bits(d) + n  (int16; 0 exactly at max positions)
                    cand = bp.tile([P, T * 64], i16, name="cand")
                    v.tensor_tensor(
                        cand[:], dsub[:].bitcast(i16), idxi_full[:], Op.add
                    )

                    # per-field argmax position, int16 (first occurrence)
                    v.tensor_reduce(
                        am4[:, ci * T : (ci + 1) * T, :],
                        cand.rearrange("p (t g n) -> p t g n", g=4, n=16),
                        axis=X,
                        op=Op.min,
                    )
                    # flags (>= 0.5) for cols 0..3 as int16 0/1
                    v.tensor_scalar(
                        fl4[:, ci * T : (ci + 1) * T, :],
                        x3[:, :, 0:4],
                        0.5,
                        None,
                        Op.is_ge,
                    )

                # --- per-token algebra for the group, all int16 ------------
                mk = fl4[:, :, 0:1]
                ia = fl4[:, :, 1:2]
                io = fl4[:, :, 2:3]
                ix = fl4[:, :, 3:4]

                def t1(nm):
                    t_ = sp.tile([P, GT], i16, name=nm)
                    return t_.unsqueeze(2)   # [P, GT, 1]

                alpha = t1("alpha")          # 1 - is_and
                v.tensor_scalar(alpha, ia, -1.0, 1.0, Op.mult, Op.add)
                s1 = t1("s1")                # 3 - is_or
                v.tensor_scalar(s1, io, -1.0, 3.0, Op.mult, Op.add)
                s3 = t1("s3")                # is_or - 2
                v.tensor_scalar(s3, io, -2.0, None, Op.add)
                s2 = t1("s2")
                v.tensor_tensor(s2, ia, s1, Op.mult)
                beta = t1("beta")            # 1 / -1 / -2
                v.tensor_tensor(beta, s2, s3, Op.add)
                or1 = t1("or1")
                v.tensor_tensor(or1, ia, io, Op.bitwise_or)
                or2 = t1("or2")
                v.tensor_tensor(or2, or1, ix, Op.bitwise_or)
                acti = t1("acti")            # active = mark & any-flag
                v.tensor_tensor(acti, mk, or2, Op.bitwise_and)
                goff = t1("goff")            # 16*(1-active)
                v.tensor_scalar(goff, acti, -16.0, 16.0, Op.mult, Op.add)

                # per half (lo: fields 0&2, hi: fields 1&3)
                e2s = []
                for h in range(2):
                    qi = t1(f"qi{h}")        # a AND b
                    v.tensor_tensor(
                        qi, am4[:, :, h : h + 1], am4[:, :, h + 2 : h + 3],
                        Op.bitwise_and,
                    )
                    ss = t1(f"ss{h}")        # a + b
                    v.tensor_tensor(
                        ss, am4[:, :, h : h + 1], am4[:, :, h + 2 : h + 3], Op.add
                    )
                    c1 = t1(f"c1{h}")
                    v.tensor_tensor(c1, ss, alpha, Op.mult)
                    c2 = t1(f"c2{h}")
                    v.tensor_tensor(c2, qi, beta, Op.mult)
                    res = t1(f"res{h}")      # op(a, b), 0..15
                    v.tensor_tensor(res, c1, c2, Op.add)
                    resg = t1(f"resg{h}")    # pushed out of 0..15 if inactive
                    v.tensor_tensor(resg, res, goff, Op.add)

                    # one-hot: eq against 0..15, then 2x scale on ACT
                    eqh = sp.tile([P, GT * 16], bf16, name=f"eqh{h}")
                    v.tensor_tensor(
                        eqh.rearrange("p (t n) -> p t n", n=16),
                        idxi_full[:, 0 : GT * 16].rearrange(
                            "p (t n) -> p t n", n=16
                        ),
                        resg.broadcast_to([P, GT, 16]),
                        Op.is_equal,
                    )
                    e2 = sp.tile([P, GT * 16], bf16, name=f"e2{h}")
                    nc.scalar.activation(
                        e2[:], eqh[:], mybir.ActivationFunctionType.Copy,
                        bias=0.0, scale=2.0,
                    )
                    e2s.append(e2.rearrange("p (t n) -> p t n", n=16))

                # --- accumulate into x and store, per chunk ----------------
                for ci in range(group):
                    i = gi * group + ci
                    x3 = xts[ci].rearrange("p (t f) -> p t f", f=D)
                    for h, off in enumerate((OUT_LO, OUT_HI)):
                        xs = x3[:, :, off : off + 16]
                        g.tensor_tensor(
                            xs, xs, e2s[h][:, ci * T : (ci + 1) * T, :], Op.add
                        )
                    dst = y_dram[i * chunk_tok : (i + 1) * chunk_tok, :].rearrange(
                        "(p t) f -> p (t f)", p=P
                    )
                    nc.sync.dma_start(dst, xts[ci][:])

    nc.compile()
    return nc


_compiled = None


def _get_compiled():
    global _compiled
    if _compiled is None:
        _compiled = build_program()
    return _compiled


def run_on_hw(nc, shards, trace=False, **kw):
    from concourse.bass_utils import run_bass_kernel_spmd

    return run_bass_kernel_spmd(
        nc, [{"x": s} for s in shards], list(range(N_CORES)), trace=trace, **kw
    )


def kernel(x_bd, and_table=None, or_table=None, xor_table=None):
    x = np.ascontiguousarray(np.asarray(x_bd, dtype=np.float32)).reshape(TOK, D)
    shards = [
        np.ascontiguousarray(x[c * TOK_PER_CORE : (c + 1) * TOK_PER_CORE])
        for c in range(N_CORES)
    ]
    nc = _get_compiled()
    res = run_on_hw(nc, shards)
    out = np.concatenate([res.results[c]["y"] for c in range(N_CORES)], axis=0)
    return out.reshape(B, S, D).astype(np.float32)
